# revision 1
# baseline (speedup 1.0000x reference)
"""KGFIT scoring kernel for 8x Trainium2 NeuronCores (Bass/Tile).

Strategy (data-parallel, no collectives):
  - Batch rows sharded 8 ways (256 rows/core); entity tables replicated.
  - Dominant cost: gather of B*M neg rows from two [200000,512] f32 tables
    (536 MB total) -> indirect SWDGE gathers, embT accumulated into embA's
    tile during DMA (cce add), so negcomb needs no DVE add.
  - neg layout: [row-on-partition, m, D]; q2 subtract is a free-dim AP
    broadcast on DVE; |.| folded into tensor_reduce(apply_absolute_value).
  - Pairwise min-distance (inter_cluster): f32 PE matmul d2 = -2*x_i.x_j
    + sq_i + sq_j with the sq terms folded in as an extra K=2 matmul.
    Per-core column permutation puts each core's own 512 rows in columns
    0..511 so the diagonal mask (eye*1e9) is a static slice.
  - All sqrt / means / final score assembly on host (tiny).
"""

import sys
from dataclasses import dataclass

import numpy as np

sys.path.insert(0, "/opt/trn_rl_repo")

RHO, ALPHA, BETA = 0.5, 0.5, 0.5
GAMMA, GAMMA_2 = 12.0, 1.0
LAM1, LAM2 = 1.0, 1.0
EPS = 1e-12
P = 128


@dataclass(frozen=True)
class Cfg:
    nent: int = 200000
    nrel: int = 1000
    nclu: int = 10000
    npar: int = 500
    d: int = 512
    b: int = 2048
    m: int = 64
    ncores: int = 8
    mg: int = 4  # neg rows of m gathered per indirect DMA chunk

    @property
    def pc(self):  # batch rows per core
        return self.b // self.ncores

    @property
    def rc(self):  # 128-row chunks of pc
        return self.pc // P

    @property
    def hr(self):  # hier rows per core (h + t)
        return 2 * self.pc

    @property
    def htc(self):  # 128-row chunks of hr
        return self.hr // P

    @property
    def mt(self):  # 128-row mtiles of hr
        return self.hr // P

    @property
    def nall(self):  # total clus rows
        return 2 * self.b

    @property
    def jbn(self):  # 512-col j blocks
        return max(1, (self.nall + 511) // 512)

    @property
    def kc(self):  # 128-row K chunks of d
        return self.d // P

    @property
    def ng(self):  # neg gather chunks per row-chunk
        return self.m // self.mg


REAL = Cfg()

_PROG_CACHE = {}


def build_program(cfg: Cfg):
    from concourse import bacc, tile
    import concourse.bass as bass
    import concourse.mybir as mybir

    f32 = mybir.dt.float32
    i32 = mybir.dt.int32
    IOA = bass.IndirectOffsetOnAxis
    AL = mybir.AluOpType
    AX = mybir.AxisListType

    nc = bacc.Bacc(None, target_bir_lowering=False)

    # ---- DRAM tensors
    embS_d = nc.dram_tensor("embS", [cfg.nent, cfg.d], f32, kind="ExternalInput")
    sdiff_d = nc.dram_tensor("sdiff", [P, cfg.htc, cfg.d], f32, kind="ExternalInput")
    negidx_d = nc.dram_tensor("negidx", [P, cfg.rc, cfg.m], i32, kind="ExternalInput")
    htidx_d = nc.dram_tensor("htidx", [P, cfg.htc], i32, kind="ExternalInput")
    rel2_d = nc.dram_tensor("rel2", [P, cfg.rc, cfg.d], f32, kind="ExternalInput")
    clusT_d = nc.dram_tensor("clusT", [P, cfg.kc, cfg.nall], f32, kind="ExternalInput")
    lhs2_d = nc.dram_tensor("lhs2", [P, cfg.kc, cfg.hr], f32, kind="ExternalInput")
    exlhsT_d = nc.dram_tensor("exlhsT", [2, cfg.hr], f32, kind="ExternalInput")
    exrhs_d = nc.dram_tensor("exrhs", [2, cfg.nall], f32, kind="ExternalInput")
    clusr_d = nc.dram_tensor("clusr", [P, cfg.htc, cfg.d], f32, kind="ExternalInput")
    pars_d = nc.dram_tensor("pars", [P, cfg.htc, cfg.d], f32, kind="ExternalInput")
    eye_d = nc.dram_tensor("eye", [P, P], f32, kind="ExternalInput")

    oneg_d = nc.dram_tensor("o_neg", [P, cfg.rc, cfg.m], f32, kind="ExternalOutput")
    otrue_d = nc.dram_tensor("o_true", [P, cfg.rc], f32, kind="ExternalOutput")
    otext_d = nc.dram_tensor("o_text", [P, cfg.htc], f32, kind="ExternalOutput")
    ointra_d = nc.dram_tensor("o_intra", [P, cfg.htc], f32, kind="ExternalOutput")
    opar_d = nc.dram_tensor("o_par", [P, cfg.htc], f32, kind="ExternalOutput")
    ointer_d = nc.dram_tensor("o_inter", [P, cfg.mt], f32, kind="ExternalOutput")

    with tile.TileContext(nc) as tc:
        with (
            tc.tile_pool(name="const", bufs=1) as const,
            tc.tile_pool(name="work", bufs=4) as work,
            tc.tile_pool(name="cblk", bufs=3) as cblk,
            tc.tile_pool(name="small", bufs=1) as small,
            tc.tile_pool(name="psum", bufs=6, space="PSUM") as psum,
        ):
            # ---- constant loads (HWDGE; keep gpsimd free for gathers)
            negidx_sb = const.tile([P, cfg.rc, cfg.m], i32)
            nc.sync.dma_start(negidx_sb[:], negidx_d[:])
            htidx_sb = const.tile([P, cfg.htc], i32)
            nc.sync.dma_start(htidx_sb[:], htidx_d[:])
            rel2_sb = const.tile([P, cfg.rc, cfg.d], f32)
            nc.sync.dma_start(rel2_sb[:], rel2_d[:])
            lhs2_sb = const.tile([P, cfg.kc, cfg.hr], f32)
            nc.sync.dma_start(lhs2_sb[:], lhs2_d[:])
            exlhsT_sb = const.tile([2, cfg.hr], f32)
            nc.sync.dma_start(exlhsT_sb[:], exlhsT_d[:])
            exrhs_sb = const.tile([2, cfg.nall], f32)
            nc.sync.dma_start(exrhs_sb[:], exrhs_d[:])
            clusr_sb = const.tile([P, cfg.htc, cfg.d], f32)
            nc.sync.dma_start(clusr_sb[:], clusr_d[:])
            pars_sb = const.tile([P, cfg.htc, cfg.d], f32)
            nc.sync.dma_start(pars_sb[:], pars_d[:])
            eye_sb = const.tile([P, P], f32)
            nc.sync.dma_start(eye_sb[:], eye_d[:])

            # ---- prologue: h/t gathers and per-row terms
            S_ht = const.tile([P, cfg.htc, cfg.d], f32)  # A+T = 2*comb
            for c in range(cfg.htc):
                nc.gpsimd.indirect_dma_start(
                    out=S_ht[:, c, :], out_offset=None, in_=embS_d[:],
                    in_offset=IOA(ap=htidx_sb[:, c:c + 1], axis=0))
            sdiff_sb = const.tile([P, cfg.htc, cfg.d], f32)
            nc.sync.dma_start(sdiff_sb[:], sdiff_d[:])
            q2_sb = const.tile([P, cfg.rc, cfg.d], f32)  # 2*(h_comb+rel)
            nc.vector.tensor_add(q2_sb[:], S_ht[:, 0:cfg.rc, :], rel2_sb[:])

            # true: sum |q2 - S_t| (host: GAMMA - 0.5*raw)
            ttmp = small.tile([P, cfg.rc, cfg.d], f32, tag="ttmp")
            nc.vector.tensor_sub(ttmp[:], q2_sb[:], S_ht[:, cfg.rc:2 * cfg.rc, :])
            truacc = const.tile([P, cfg.rc], f32)
            nc.vector.tensor_reduce(
                truacc[:], ttmp[:], axis=AX.X, op=AL.add, apply_absolute_value=True)
            nc.sync.dma_start(otrue_d[:], truacc[:])

            # text: sum (A-T)^2  (host: sqrt(0.25*raw + EPS))
            tsq = small.tile([P, cfg.htc, cfg.d], f32, tag="tsq")
            nc.vector.tensor_mul(tsq[:], sdiff_sb[:], sdiff_sb[:])
            txacc = const.tile([P, cfg.htc], f32)
            nc.vector.tensor_reduce(txacc[:], tsq[:], axis=AX.X, op=AL.add)
            nc.sync.dma_start(otext_d[:], txacc[:])

            # intra: sum (0.5*S - clus)^2
            c2 = small.tile([P, cfg.htc, cfg.d], f32, tag="c2")
            nc.vector.scalar_tensor_tensor(
                out=c2[:], in0=S_ht[:], scalar=0.5, in1=clusr_sb[:],
                op0=AL.mult, op1=AL.subtract)
            c3 = small.tile([P, cfg.htc, cfg.d], f32, tag="c3")
            nc.vector.tensor_mul(c3[:], c2[:], c2[:])
            inacc = const.tile([P, cfg.htc], f32)
            nc.vector.tensor_reduce(inacc[:], c3[:], axis=AX.X, op=AL.add)
            nc.sync.dma_start(ointra_d[:], inacc[:])

            # parent: sum (clus - par)^2
            p1 = small.tile([P, cfg.htc, cfg.d], f32, tag="p1")
            nc.vector.tensor_sub(p1[:], clusr_sb[:], pars_sb[:])
            p2 = small.tile([P, cfg.htc, cfg.d], f32, tag="p2")
            nc.vector.tensor_mul(p2[:], p1[:], p1[:])
            pacc = const.tile([P, cfg.htc], f32)
            nc.vector.tensor_reduce(pacc[:], p2[:], axis=AX.X, op=AL.add)
            nc.sync.dma_start(opar_d[:], pacc[:])

            # ---- NEG phase (the 536MB gather): program order first = priority
            negacc = const.tile([P, cfg.rc, cfg.m], f32)
            for rcb in range(cfg.rc):
                q2s = q2_sb[:, rcb, :]
                q2bc = bass.AP(
                    q2s.tensor, q2s.offset,
                    [q2s.ap[0], [0, cfg.mg], q2s.ap[1]])  # [P, mg, d] bcast
                for g in range(cfg.ng):
                    at = work.tile([P, cfg.mg, cfg.d], f32, tag="negload")
                    for j in range(cfg.mg):
                        ix = negidx_sb[:, rcb, g * cfg.mg + j:g * cfg.mg + j + 1]
                        nc.gpsimd.indirect_dma_start(
                            out=at[:, j, :], out_offset=None, in_=embS_d[:],
                            in_offset=IOA(ap=ix, axis=0))
                    nc.vector.tensor_sub(at[:], at[:], q2bc)
                    nc.vector.tensor_reduce(
                        negacc[:, rcb, g * cfg.mg:(g + 1) * cfg.mg], at[:],
                        axis=AX.X, op=AL.add, apply_absolute_value=True)
            nc.sync.dma_start(oneg_d[:], negacc[:])

            # ---- PAIRWISE phase (PE, overlaps neg DMA). clusT streamed by
            # 512-col block; per-(jb,mt) block-min collected in jmall.
            nslot = cfg.jbn + 2
            jmall = const.tile([P, cfg.mt, nslot], f32)
            nc.vector.memset(jmall[:], 1e30)
            ointer_sb = const.tile([P, cfg.mt], f32)
            for jb in range(cfg.jbn):
                w = min(512, cfg.nall - jb * 512)
                cblk_sb = cblk.tile([P, cfg.kc, 512], f32, tag="cblk")
                nc.sync.dma_start(
                    cblk_sb[:, :, 0:w], clusT_d[:, :, jb * 512:jb * 512 + w])
                for mt in range(cfg.mt):
                    ms = slice(mt * P, (mt + 1) * P)
                    pw = psum.tile([P, w], f32, tag="pw", name=f"pw_{jb}_{mt}")
                    for kcb in range(cfg.kc):
                        nc.tensor.matmul(
                            pw[:], lhsT=lhs2_sb[:, kcb, ms],
                            rhs=cblk_sb[:, kcb, 0:w],
                            start=(kcb == 0), stop=False)
                    nc.tensor.matmul(
                        pw[:], lhsT=exlhsT_sb[:, ms],
                        rhs=exrhs_sb[:, jb * 512:jb * 512 + w],
                        start=False, stop=True)
                    if jb == 0:
                        # diag block: own cols are 0..hr-1 (perm puts own first)
                        ysb = small.tile([P, P], f32, tag="ydiag")
                        nc.vector.tensor_add(ysb[:], pw[:, ms], eye_sb[:])
                        nc.vector.tensor_reduce(
                            jmall[:, mt, 0:1], ysb[:], axis=AX.X, op=AL.min)
                        if mt > 0:
                            nc.vector.tensor_reduce(
                                jmall[:, mt, 1:2], pw[:, 0:mt * P],
                                axis=AX.X, op=AL.min)
                        if (mt + 1) * P < w:
                            nc.vector.tensor_reduce(
                                jmall[:, mt, 2:3], pw[:, (mt + 1) * P:w],
                                axis=AX.X, op=AL.min)
                    else:
                        nc.vector.tensor_reduce(
                            jmall[:, mt, 2 + jb:3 + jb], pw[:],
                            axis=AX.X, op=AL.min)
            for mt in range(cfg.mt):
                nc.vector.tensor_reduce(
                    ointer_sb[:, mt:mt + 1], jmall[:, mt, :], axis=AX.X, op=AL.min)
            nc.sync.dma_start(ointer_d[:], ointer_sb[:])

    nc.compile()
    return nc


def _chunked(x, nch):
    """[N, ...] -> [128, nch, ...] with row r at [r%128, r//128]."""
    n = x.shape[0]
    assert n == nch * P
    return np.ascontiguousarray(x.reshape(nch, P, *x.shape[1:]).transpose(
        1, 0, *range(2, x.ndim + 1)))


def make_in_maps(cfg: Cfg, sample, neg_tails, cluster_assign, parent_assign,
                 relation_embedding, entity_embedding_init,
                 entity_text_embeddings, cluster_emb, parent_emb):
    f4 = np.float32
    sample = np.asarray(sample)
    neg_tails = np.asarray(neg_tails)
    cluster_assign = np.asarray(cluster_assign)
    parent_assign = np.asarray(parent_assign)
    relation_embedding = np.asarray(relation_embedding, dtype=f4)
    embA = np.asarray(entity_embedding_init, dtype=f4)
    embT = np.asarray(entity_text_embeddings, dtype=f4)
    embS = embA + embT
    embD = embA - embT
    cluster_emb = np.asarray(cluster_emb, dtype=f4)
    parent_emb = np.asarray(parent_emb, dtype=f4)

    h_all = sample[:, 0].astype(np.int64)
    t_all = sample[:, 2].astype(np.int64)
    cid_all = cluster_assign[np.concatenate([h_all, t_all])]
    clus = cluster_emb[cid_all]  # [2B, d]
    sq_all = np.sum(clus * clus, axis=1, dtype=f4)
    pars_all = parent_emb[parent_assign[cid_all]]
    eye = (np.eye(P) * 1e9).astype(f4)

    in_maps, aux = [], []
    for k in range(cfg.ncores):
        bs = slice(k * cfg.pc, (k + 1) * cfg.pc)
        h = sample[bs, 0].astype(np.int64)
        r = (sample[bs, 1] % cfg.nrel).astype(np.int64)
        t = sample[bs, 2].astype(np.int64)
        neg = neg_tails[bs].astype(np.int32)  # [pc, m]
        negidx = _chunked(neg, cfg.rc)
        htrows = np.concatenate([h, t]).astype(np.int32)
        htidx = _chunked(htrows, cfg.htc)
        rel2 = (2.0 * relation_embedding[r]).astype(f4)
        rel2_in = _chunked(rel2, cfg.rc)

        own = np.concatenate([np.arange(k * cfg.pc, (k + 1) * cfg.pc),
                              np.arange(cfg.b + k * cfg.pc,
                                        cfg.b + (k + 1) * cfg.pc)])
        mask = np.ones(cfg.nall, dtype=bool)
        mask[own] = False
        perm = np.concatenate([own, np.nonzero(mask)[0]])
        clusP = clus[perm]
        sqP = sq_all[perm]
        clusT_in = np.ascontiguousarray(
            clusP.T.reshape(cfg.kc, P, cfg.nall).transpose(1, 0, 2))
        lhs2_in = np.ascontiguousarray(
            (-2.0 * clusP[:cfg.hr]).T.reshape(cfg.kc, P, cfg.hr).transpose(1, 0, 2))
        exlhsT = np.stack([np.ones(cfg.hr, f4), sqP[:cfg.hr]]).astype(f4)
        exrhs = np.stack([sqP, np.ones(cfg.nall, f4)]).astype(f4)
        clusr_in = _chunked(clus[own].astype(f4), cfg.htc)
        pars_in = _chunked(pars_all[own].astype(f4), cfg.htc)

        sdiff_in = _chunked(embD[htrows.astype(np.int64)], cfg.htc)
        in_maps.append({
            "embS": embS, "sdiff": sdiff_in,
            "negidx": negidx.astype(np.int32), "htidx": htidx.astype(np.int32),
            "rel2": rel2_in, "clusT": clusT_in.astype(f4),
            "lhs2": lhs2_in.astype(f4), "exlhsT": exlhsT, "exrhs": exrhs,
            "clusr": clusr_in, "pars": pars_in, "eye": eye,
        })
        aux.append({})
    return in_maps, aux


def _unchunk(x):
    """[128, nch, ...] -> [nch*128, ...] inverting _chunked."""
    return np.ascontiguousarray(
        x.transpose(1, 0, *range(2, x.ndim))).reshape(-1, *x.shape[2:])


def assemble(cfg: Cfg, results):
    f4 = np.float32
    true_s, hd, td, mean_neg = [], [], [], []
    intra_d, par_d, inter_d = [], [], []
    for k in range(cfg.ncores):
        r = results[k]
        raw_neg = _unchunk(r["o_neg"])  # [pc, m]
        neg_scores = (GAMMA - 0.5 * raw_neg).astype(f4)
        mean_neg.append(neg_scores.mean(axis=1, dtype=f4))
        raw_true = _unchunk(r["o_true"][:, :, None])[:, 0]  # [pc]
        true_s.append((GAMMA - 0.5 * raw_true).astype(f4))
        text_sq = _unchunk(r["o_text"][:, :, None])[:, 0]  # [hr]
        text_d = np.sqrt(0.25 * text_sq + EPS, dtype=f4)
        hd.append(text_d[:cfg.pc])
        td.append(text_d[cfg.pc:])
        intra_sq = _unchunk(r["o_intra"][:, :, None])[:, 0]
        intra_d.append(np.sqrt(intra_sq + EPS, dtype=f4))
        par_sq = _unchunk(r["o_par"][:, :, None])[:, 0]
        par_d.append(np.sqrt(par_sq + EPS, dtype=f4))
        inter_min = _unchunk(r["o_inter"][:, :, None])[:, 0]
        inter_d.append(np.sqrt(np.maximum(inter_min, EPS), dtype=f4))

    intra_loss = np.concatenate(intra_d).mean(dtype=f4)
    inter_loss = np.concatenate(inter_d).mean(dtype=f4)
    par_loss = np.concatenate(par_d).mean(dtype=f4)
    hier = intra_loss - LAM1 * inter_loss + LAM2 * par_loss

    true_s = np.concatenate(true_s)
    hd = np.concatenate(hd)
    td = np.concatenate(td)
    mean_neg = np.concatenate(mean_neg)
    score = (-ALPHA * hier - BETA * (hd + td)
             - GAMMA_2 * (true_s - mean_neg)).astype(f4)
    return score


def run_on_device(cfg: Cfg, in_maps, trace=False):
    from concourse.bass_utils import run_bass_kernel_spmd
    key = cfg
    if key not in _PROG_CACHE:
        _PROG_CACHE[key] = build_program(cfg)
    nc = _PROG_CACHE[key]
    res = run_bass_kernel_spmd(
        nc, in_maps, core_ids=list(range(cfg.ncores)), trace=trace)
    return res


def kernel(**inputs):
    cfg = REAL
    in_maps, _ = make_in_maps(cfg, **inputs)
    res = run_on_device(cfg, in_maps)
    return assemble(cfg, res.results)



# revision 3
# speedup vs baseline: 1.6917x; 1.6917x over previous
"""KGFIT scoring kernel for 8x Trainium2 NeuronCores (Bass/Tile).

Strategy (data-parallel, no collectives):
  - Batch rows sharded 8 ways (256 rows/core); entity table replicated in
    bf16; all O(B*D) per-row score terms (true/text/intra/parent) are
    computed on host (~3M flops) so the device only runs the two heavy
    parts: the B*M neg-row gather-reduce and the [2B,2B] pairwise min.
  - NEG phase: 8 batched indirect SWDGE gathers per core (one per
    [128,16,512] bf16 chunk) amortize the ~1us/call descriptor-gen cost;
    DVE does the broadcast subtract at 2x (bf16); the |.|-sum reduce is
    split between DVE tensor_reduce and Scalar-engine Abs+accumulate to
    balance engine load.
  - PAIRWISE phase: bf16 PE matmul pw = -2*x_i.x_j + sq_j (sq_j folded in
    as a K=2 matmul with an exact hi/lo bf16 split; sq_i added on host
    after the min so duplicate-cluster pairs stay ~0). Per-core column
    permutation puts own rows first so the diagonal mask is a static
    slice. Row-min on DVE; host adds sq_i, sqrt, means.
"""

import sys
from dataclasses import dataclass

import numpy as np

sys.path.insert(0, "/opt/trn_rl_repo")

F16 = np.float16

RHO, ALPHA, BETA = 0.5, 0.5, 0.5
GAMMA, GAMMA_2 = 12.0, 1.0
LAM1, LAM2 = 1.0, 1.0
EPS = 1e-12
P = 128


@dataclass(frozen=True)
class Cfg:
    nent: int = 200000
    nrel: int = 1000
    nclu: int = 10000
    npar: int = 500
    d: int = 512
    b: int = 2048
    m: int = 64
    ncores: int = 8
    mg: int = 16     # neg rows gathered per indirect DMA call
    nact: int = 6    # of the rc*ng neg chunks, how many reduce on ACT

    @property
    def pc(self):    # batch rows per core
        return self.b // self.ncores

    @property
    def rc(self):    # 128-row chunks of pc
        return self.pc // P

    @property
    def hr(self):    # pairwise rows per core (h + t)
        return 2 * self.pc

    @property
    def mt(self):    # 128-row mtiles of hr
        return self.hr // P

    @property
    def nall(self):  # total pairwise columns
        return 2 * self.b

    @property
    def jbn(self):   # 512-col j blocks
        return max(1, (self.nall + 511) // 512)

    @property
    def kc(self):    # 128-row K chunks of d
        return self.d // P

    @property
    def ng(self):    # neg gather chunks per row-chunk
        return self.m // self.mg


REAL = Cfg()

_PROG_CACHE = {}


def build_program(cfg: Cfg):
    from concourse import bacc, tile
    import concourse.bass as bass
    import concourse.mybir as mybir

    f32 = mybir.dt.float32
    f16 = mybir.dt.float16
    i32 = mybir.dt.int32
    IOA = bass.IndirectOffsetOnAxis
    AL = mybir.AluOpType
    AX = mybir.AxisListType
    ABS = mybir.ActivationFunctionType.Abs

    nc = bacc.Bacc(None, target_bir_lowering=False)

    # ---- DRAM tensors
    embS16_d = nc.dram_tensor("embS16", [cfg.nent, cfg.d], f16, kind="ExternalInput")
    negidx_d = nc.dram_tensor("negidx", [P, cfg.rc, cfg.m], i32, kind="ExternalInput")
    q2_d = nc.dram_tensor("q2", [P, cfg.rc, cfg.d], f16, kind="ExternalInput")
    clusT_d = nc.dram_tensor("clusT", [P, cfg.kc, cfg.nall], f16, kind="ExternalInput")
    lhs2_d = nc.dram_tensor("lhs2", [P, cfg.kc, cfg.hr], f16, kind="ExternalInput")
    ones2_d = nc.dram_tensor("ones2", [2, cfg.hr], f16, kind="ExternalInput")
    sqrows_d = nc.dram_tensor("sqrows", [2, cfg.nall], f16, kind="ExternalInput")
    eye_d = nc.dram_tensor("eye", [P, P], f32, kind="ExternalInput")

    oneg_d = nc.dram_tensor("o_neg", [P, cfg.rc, cfg.m], f32, kind="ExternalOutput")
    ointer_d = nc.dram_tensor("o_inter", [P, cfg.mt], f32, kind="ExternalOutput")

    nchunks = cfg.rc * cfg.ng
    # chunks reduced on ACT (rest on DVE), spread across the phase
    act_set = set(range(nchunks - cfg.nact, nchunks)) if cfg.nact else set()

    with tile.TileContext(nc) as tc:
        with (
            tc.tile_pool(name="const", bufs=1) as const,
            tc.tile_pool(name="work", bufs=3) as work,
            tc.tile_pool(name="dwork", bufs=3) as dwork,
            tc.tile_pool(name="twork", bufs=2) as twork,
            tc.tile_pool(name="cblk", bufs=3) as cblk,
            tc.tile_pool(name="small", bufs=1) as small,
            tc.tile_pool(name="psum", bufs=6, space="PSUM") as psum,
        ):
            # ---- constant loads (HWDGE)
            negidx_sb = const.tile([P, cfg.rc, cfg.m], i32)
            nc.sync.dma_start(negidx_sb[:], negidx_d[:])
            q2_sb = const.tile([P, cfg.rc, cfg.d], f16)
            nc.sync.dma_start(q2_sb[:], q2_d[:])
            lhs2_sb = const.tile([P, cfg.kc, cfg.hr], f16)
            nc.sync.dma_start(lhs2_sb[:], lhs2_d[:])
            ones2_sb = const.tile([2, cfg.hr], f16)
            nc.sync.dma_start(ones2_sb[:], ones2_d[:])
            sqrows_sb = const.tile([2, cfg.nall], f16)
            nc.sync.dma_start(sqrows_sb[:], sqrows_d[:])
            eye_sb = const.tile([P, P], f32)
            nc.sync.dma_start(eye_sb[:], eye_d[:])

            # ---- NEG phase: batched gathers, bf16 sub, split reduce
            negacc = const.tile([P, cfg.rc, cfg.m], f32)
            for rcb in range(cfg.rc):
                q2s = q2_sb[:, rcb, :]
                q2bc = bass.AP(
                    q2s.tensor, q2s.offset,
                    [q2s.ap[0], [0, cfg.mg], q2s.ap[1]])  # [P, mg, d] bcast
                for g in range(cfg.ng):
                    c = rcb * cfg.ng + g
                    ms = slice(g * cfg.mg, (g + 1) * cfg.mg)
                    at = work.tile([P, cfg.mg, cfg.d], f16, tag="negload")
                    for j in range(cfg.mg):
                        mj = g * cfg.mg + j
                        nc.gpsimd.indirect_dma_start(
                            out=at[:, j, :], out_offset=None, in_=embS16_d[:],
                            in_offset=IOA(ap=negidx_sb[:, rcb, mj:mj + 1], axis=0))
                    diff = dwork.tile([P, cfg.mg, cfg.d], f16, tag="diff")
                    nc.vector.tensor_sub(diff[:], at[:], q2bc)
                    if c in act_set:
                        trash = twork.tile([P, cfg.mg, cfg.d], f16, tag="trash")
                        for j in range(cfg.mg):
                            mj = g * cfg.mg + j
                            nc.scalar.activation(
                                out=trash[:, j, :], in_=diff[:, j, :], func=ABS,
                                accum_out=negacc[:, rcb, mj:mj + 1])
                    else:
                        nc.vector.tensor_reduce(
                            negacc[:, rcb, ms], diff[:],
                            axis=AX.X, op=AL.add, apply_absolute_value=True)
            nc.sync.dma_start(oneg_d[:], negacc[:])

            # ---- PAIRWISE phase (PE bf16; overlaps neg DMA/DVE/ACT)
            nslot = cfg.jbn + 2
            jmall = const.tile([P, cfg.mt, nslot], f32)
            nc.vector.memset(jmall[:], 1e30)
            ointer_sb = const.tile([P, cfg.mt], f32)
            for jb in range(cfg.jbn):
                w = min(512, cfg.nall - jb * 512)
                cblk_sb = cblk.tile([P, cfg.kc, 512], f16, tag="cblk")
                nc.sync.dma_start(
                    cblk_sb[:, :, 0:w], clusT_d[:, :, jb * 512:jb * 512 + w])
                for mt in range(cfg.mt):
                    ms = slice(mt * P, (mt + 1) * P)
                    pw = psum.tile([P, w], f32, tag="pw", name=f"pw_{jb}_{mt}")
                    for kcb in range(cfg.kc):
                        nc.tensor.matmul(
                            pw[:], lhsT=lhs2_sb[:, kcb, ms],
                            rhs=cblk_sb[:, kcb, 0:w],
                            start=(kcb == 0), stop=False)
                    nc.tensor.matmul(
                        pw[:], lhsT=ones2_sb[:, ms],
                        rhs=sqrows_sb[:, jb * 512:jb * 512 + w],
                        start=False, stop=True)
                    if jb == 0:
                        # diag block: own cols 0..hr-1 (perm puts own first)
                        ysb = small.tile([P, P], f32, tag="ydiag")
                        nc.vector.tensor_add(ysb[:], pw[:, ms], eye_sb[:])
                        nc.vector.tensor_reduce(
                            jmall[:, mt, 0:1], ysb[:], axis=AX.X, op=AL.min)
                        if mt > 0:
                            nc.vector.tensor_reduce(
                                jmall[:, mt, 1:2], pw[:, 0:mt * P],
                                axis=AX.X, op=AL.min)
                        if (mt + 1) * P < w:
                            nc.vector.tensor_reduce(
                                jmall[:, mt, 2:3], pw[:, (mt + 1) * P:w],
                                axis=AX.X, op=AL.min)
                    else:
                        nc.vector.tensor_reduce(
                            jmall[:, mt, 2 + jb:3 + jb], pw[:],
                            axis=AX.X, op=AL.min)
            for mt in range(cfg.mt):
                nc.vector.tensor_reduce(
                    ointer_sb[:, mt:mt + 1], jmall[:, mt, :], axis=AX.X, op=AL.min)
            nc.sync.dma_start(ointer_d[:], ointer_sb[:])

    nc.compile()
    return nc


def _chunked(x, nch):
    """[N, ...] -> [128, nch, ...] with row r at [r%128, r//128]."""
    n = x.shape[0]
    assert n == nch * P
    return np.ascontiguousarray(x.reshape(nch, P, *x.shape[1:]).transpose(
        1, 0, *range(2, x.ndim + 1)))


def make_in_maps(cfg: Cfg, sample, neg_tails, cluster_assign, parent_assign,
                 relation_embedding, entity_embedding_init,
                 entity_text_embeddings, cluster_emb, parent_emb):
    f4 = np.float32
    sample = np.asarray(sample)
    neg_tails = np.asarray(neg_tails)
    cluster_assign = np.asarray(cluster_assign)
    parent_assign = np.asarray(parent_assign)
    relation_embedding = np.asarray(relation_embedding, dtype=f4)
    embA = np.asarray(entity_embedding_init, dtype=f4)
    embT = np.asarray(entity_text_embeddings, dtype=f4)
    embS = embA + embT          # = 2 * comb
    embS16 = embS.astype(F16)
    cluster_emb = np.asarray(cluster_emb, dtype=f4)
    parent_emb = np.asarray(parent_emb, dtype=f4)

    h_all = sample[:, 0].astype(np.int64)
    r_all = (sample[:, 1] % cfg.nrel).astype(np.int64)
    t_all = sample[:, 2].astype(np.int64)
    ht_all = np.concatenate([h_all, t_all])
    cid_all = cluster_assign[ht_all]
    clus = cluster_emb[cid_all]                       # [2B, d] f32
    clus16 = clus.astype(F16)                        # device-consistent rounding
    clus16f = clus16.astype(f4)
    sq_all = np.sum(clus16f * clus16f, axis=1, dtype=f4)   # [2B] from bf16 vals
    sq_hi = sq_all.astype(F16)
    sq_lo = (sq_all - sq_hi.astype(f4)).astype(F16)
    pars_all = parent_emb[parent_assign[cid_all]]
    eye = (np.eye(P) * 1e9).astype(f4)

    # ---- host-side per-row score terms (exact f32)
    Sh, St = embS[h_all], embS[t_all]
    rel = relation_embedding[r_all]
    true_s = (GAMMA - np.abs(0.5 * Sh + rel - 0.5 * St).sum(axis=1, dtype=f4)
              ).astype(f4)                                        # [B]
    embD_ht = embA[ht_all] - embT[ht_all]
    text_d = np.sqrt(0.25 * np.sum(embD_ht * embD_ht, axis=1, dtype=f4) + EPS)
    intra_v = 0.5 * embS[ht_all] - clus
    intra_d = np.sqrt(np.sum(intra_v * intra_v, axis=1, dtype=f4) + EPS)
    par_v = clus - pars_all
    par_d = np.sqrt(np.sum(par_v * par_v, axis=1, dtype=f4) + EPS)
    host = {
        "true_s": true_s,
        "hd": text_d[:cfg.b].astype(f4),
        "td": text_d[cfg.b:].astype(f4),
        "intra_loss": intra_d.mean(dtype=f4),
        "par_loss": par_d.mean(dtype=f4),
        "sq_all": sq_all,
    }

    in_maps = []
    perms = []
    for k in range(cfg.ncores):
        bs = slice(k * cfg.pc, (k + 1) * cfg.pc)
        h = h_all[bs]
        r = r_all[bs]
        neg = neg_tails[bs].astype(np.int32)          # [pc, m]
        negidx = _chunked(neg, cfg.rc)
        q2 = (embS[h] + 2.0 * relation_embedding[r]).astype(F16)
        q2_in = _chunked(q2, cfg.rc)

        own = np.concatenate([np.arange(k * cfg.pc, (k + 1) * cfg.pc),
                              np.arange(cfg.b + k * cfg.pc,
                                        cfg.b + (k + 1) * cfg.pc)])
        mask = np.ones(cfg.nall, dtype=bool)
        mask[own] = False
        perm = np.concatenate([own, np.nonzero(mask)[0]])
        perms.append(perm)
        clusP = clus16[perm]                          # [nall, d] bf16
        clusT_in = np.ascontiguousarray(
            clusP.T.reshape(cfg.kc, P, cfg.nall).transpose(1, 0, 2))
        lhs2_in = np.ascontiguousarray(
            (-2.0 * clusP[:cfg.hr].astype(f4)).astype(F16)
            .T.reshape(cfg.kc, P, cfg.hr).transpose(1, 0, 2))
        ones2 = np.ones((2, cfg.hr), dtype=F16)
        sqrows = np.stack([sq_hi[perm], sq_lo[perm]]).astype(F16)

        in_maps.append({
            "embS16": embS16,
            "negidx": negidx.astype(np.int32),
            "q2": q2_in,
            "clusT": clusT_in,
            "lhs2": lhs2_in,
            "ones2": ones2,
            "sqrows": sqrows,
            "eye": eye,
        })
    return in_maps, (host, perms)


def _unchunk(x):
    """[128, nch, ...] -> [nch*128, ...] inverting _chunked."""
    return np.ascontiguousarray(
        x.transpose(1, 0, *range(2, x.ndim))).reshape(-1, *x.shape[2:])


def assemble(cfg: Cfg, results, aux):
    host, perms = aux
    f4 = np.float32
    mean_neg, inter_d2 = [], np.empty(cfg.nall, dtype=f4)
    for k in range(cfg.ncores):
        r = results[k]
        raw_neg = _unchunk(r["o_neg"])                # [pc, m]
        neg_scores = (GAMMA - 0.5 * raw_neg).astype(f4)
        mean_neg.append(neg_scores.mean(axis=1, dtype=f4))
        own = perms[k][:cfg.hr]
        inter_min = _unchunk(r["o_inter"][:, :, None])[:, 0]   # [hr]
        inter_d2[own] = inter_min + host["sq_all"][own]
    mean_neg = np.concatenate(mean_neg)

    inter_d = np.sqrt(np.maximum(inter_d2, EPS), dtype=f4)
    inter_loss = inter_d.mean(dtype=f4)
    hier = host["intra_loss"] - LAM1 * inter_loss + LAM2 * host["par_loss"]

    score = (-ALPHA * hier - BETA * (host["hd"] + host["td"])
             - GAMMA_2 * (host["true_s"] - mean_neg)).astype(f4)
    return score


def run_on_device(cfg: Cfg, in_maps, trace=False):
    from concourse.bass_utils import run_bass_kernel_spmd
    key = cfg
    if key not in _PROG_CACHE:
        _PROG_CACHE[key] = build_program(cfg)
    nc = _PROG_CACHE[key]
    res = run_bass_kernel_spmd(
        nc, in_maps, core_ids=list(range(cfg.ncores)), trace=trace)
    return res


def kernel(**inputs):
    cfg = REAL
    in_maps, aux = make_in_maps(cfg, **inputs)
    res = run_on_device(cfg, in_maps)
    return assemble(cfg, res.results, aux)


# revision 4
# speedup vs baseline: 1.7922x; 1.0595x over previous
"""KGFIT scoring kernel for 8x Trainium2 NeuronCores (Bass/Tile).

Strategy (data-parallel, no collectives):
  - Batch rows sharded 8 ways (256 rows/core); entity table replicated in
    bf16; all O(B*D) per-row score terms (true/text/intra/parent) are
    computed on host (~3M flops) so the device only runs the two heavy
    parts: the B*M neg-row gather-reduce and the [2B,2B] pairwise min.
  - NEG phase: 8 batched indirect SWDGE gathers per core (one per
    [128,16,512] bf16 chunk) amortize the ~1us/call descriptor-gen cost;
    DVE does the broadcast subtract at 2x (bf16); the |.|-sum reduce is
    split between DVE tensor_reduce and Scalar-engine Abs+accumulate to
    balance engine load.
  - PAIRWISE phase: bf16 PE matmul pw = -2*x_i.x_j + sq_j (sq_j folded in
    as a K=2 matmul with an exact hi/lo bf16 split; sq_i added on host
    after the min so duplicate-cluster pairs stay ~0). Per-core column
    permutation puts own rows first so the diagonal mask is a static
    slice. Row-min on DVE; host adds sq_i, sqrt, means.
"""

import sys
from dataclasses import dataclass

import numpy as np

sys.path.insert(0, "/opt/trn_rl_repo")

F16 = np.float16

RHO, ALPHA, BETA = 0.5, 0.5, 0.5
GAMMA, GAMMA_2 = 12.0, 1.0
LAM1, LAM2 = 1.0, 1.0
EPS = 1e-12
P = 128


@dataclass(frozen=True)
class Cfg:
    nent: int = 200000
    nrel: int = 1000
    nclu: int = 10000
    npar: int = 500
    d: int = 512
    b: int = 2048
    m: int = 64
    ncores: int = 8
    mg: int = 8      # neg rows per processing chunk (1 indirect DMA per row)
    nact: int = 12   # of the rc*ng neg chunks, how many reduce on ACT

    @property
    def pc(self):    # batch rows per core
        return self.b // self.ncores

    @property
    def rc(self):    # 128-row chunks of pc
        return self.pc // P

    @property
    def hr(self):    # pairwise rows per core (h + t)
        return 2 * self.pc

    @property
    def mt(self):    # 128-row mtiles of hr
        return self.hr // P

    @property
    def nall(self):  # total pairwise columns
        return 2 * self.b

    @property
    def jbn(self):   # 512-col j blocks
        return max(1, (self.nall + 511) // 512)

    @property
    def kc(self):    # 128-row K chunks of d
        return self.d // P

    @property
    def ng(self):    # neg gather chunks per row-chunk
        return self.m // self.mg


REAL = Cfg()

_PROG_CACHE = {}


def build_program(cfg: Cfg):
    from concourse import bacc, tile
    import concourse.bass as bass
    import concourse.mybir as mybir

    f32 = mybir.dt.float32
    f16 = mybir.dt.float16
    i32 = mybir.dt.int32
    IOA = bass.IndirectOffsetOnAxis
    AL = mybir.AluOpType
    AX = mybir.AxisListType
    ABS = mybir.ActivationFunctionType.Abs

    nc = bacc.Bacc(None, target_bir_lowering=False)

    # ---- DRAM tensors
    embS16_d = nc.dram_tensor("embS16", [cfg.nent, cfg.d], f16, kind="ExternalInput")
    negidx_d = nc.dram_tensor("negidx", [P, cfg.rc, cfg.m], i32, kind="ExternalInput")
    q2_d = nc.dram_tensor("q2", [P, cfg.rc, cfg.d], f16, kind="ExternalInput")
    clusT_d = nc.dram_tensor("clusT", [P, cfg.kc, cfg.nall], f16, kind="ExternalInput")
    lhs2_d = nc.dram_tensor("lhs2", [P, cfg.kc, cfg.hr], f16, kind="ExternalInput")
    ones2_d = nc.dram_tensor("ones2", [2, cfg.hr], f16, kind="ExternalInput")
    sqrows_d = nc.dram_tensor("sqrows", [2, cfg.nall], f16, kind="ExternalInput")
    eye_d = nc.dram_tensor("eye", [P, P], f32, kind="ExternalInput")

    oneg_d = nc.dram_tensor("o_neg", [P, cfg.rc, cfg.m], f32, kind="ExternalOutput")
    ointer_d = nc.dram_tensor("o_inter", [P, cfg.mt], f32, kind="ExternalOutput")

    nchunks = cfg.rc * cfg.ng
    # chunks reduced on ACT (rest on DVE), spread across the phase
    ndve = nchunks - cfg.nact
    dve_set = set(round(i * (nchunks - 1) / max(ndve - 1, 1)) for i in range(ndve)) \
        if ndve else set()
    act_set = set(range(nchunks)) - dve_set

    with tile.TileContext(nc) as tc:
        with (
            tc.tile_pool(name="const", bufs=1) as const,
            tc.tile_pool(name="work", bufs=6) as work,
            tc.tile_pool(name="dwork", bufs=3) as dwork,
            tc.tile_pool(name="twork", bufs=3) as twork,
            tc.tile_pool(name="cblk", bufs=3) as cblk,
            tc.tile_pool(name="small", bufs=1) as small,
            tc.tile_pool(name="psum", bufs=8, space="PSUM") as psum,
        ):
            # ---- constant loads (HWDGE)
            negidx_sb = const.tile([P, cfg.rc, cfg.m], i32)
            nc.sync.dma_start(negidx_sb[:], negidx_d[:])
            q2_sb = const.tile([P, cfg.rc, cfg.d], f16)
            nc.sync.dma_start(q2_sb[:], q2_d[:])
            lhs2_sb = const.tile([P, cfg.kc, cfg.hr], f16)
            nc.sync.dma_start(lhs2_sb[:], lhs2_d[:])
            ones2_sb = const.tile([2, cfg.hr], f16)
            nc.sync.dma_start(ones2_sb[:], ones2_d[:])
            sqrows_sb = const.tile([2, cfg.nall], f16)
            nc.sync.dma_start(sqrows_sb[:], sqrows_d[:])
            eye_sb = const.tile([P, P], f32)
            nc.sync.dma_start(eye_sb[:], eye_d[:])

            # ---- pairwise state + unit generator (interleaved with neg)
            nslot = cfg.jbn + 2
            jmall = const.tile([P, cfg.mt, nslot], f32)
            nc.vector.memset(jmall[:], 1e30)
            ointer_sb = const.tile([P, cfg.mt], f32)
            cblk_cur = [None]

            def pw_unit(jb, mt):
                w = min(512, cfg.nall - jb * 512)
                if mt == 0:
                    cblk_sb = cblk.tile([P, cfg.kc, 512], f16, tag="cblk")
                    nc.sync.dma_start(
                        cblk_sb[:, :, 0:w], clusT_d[:, :, jb * 512:jb * 512 + w])
                    cblk_cur[0] = cblk_sb
                cblk_sb = cblk_cur[0]
                ms = slice(mt * P, (mt + 1) * P)
                pw = psum.tile([P, w], f32, tag="pw", name=f"pw_{jb}_{mt}")
                for kcb in range(cfg.kc):
                    nc.tensor.matmul(
                        pw[:], lhsT=lhs2_sb[:, kcb, ms],
                        rhs=cblk_sb[:, kcb, 0:w],
                        start=(kcb == 0), stop=False)
                nc.tensor.matmul(
                    pw[:], lhsT=ones2_sb[:, ms],
                    rhs=sqrows_sb[:, jb * 512:jb * 512 + w],
                    start=False, stop=True)
                if jb == 0:
                    # diag block: own cols 0..hr-1 (perm puts own first)
                    ysb = small.tile([P, P], f32, tag="ydiag")
                    nc.vector.tensor_add(ysb[:], pw[:, ms], eye_sb[:])
                    nc.vector.tensor_reduce(
                        jmall[:, mt, 0:1], ysb[:], axis=AX.X, op=AL.min)
                    if mt > 0:
                        nc.vector.tensor_reduce(
                            jmall[:, mt, 1:2], pw[:, 0:mt * P],
                            axis=AX.X, op=AL.min)
                    if (mt + 1) * P < w:
                        nc.vector.tensor_reduce(
                            jmall[:, mt, 2:3], pw[:, (mt + 1) * P:w],
                            axis=AX.X, op=AL.min)
                else:
                    nc.vector.tensor_reduce(
                        jmall[:, mt, 2 + jb:3 + jb], pw[:],
                        axis=AX.X, op=AL.min)

            pw_units = [(jb, mt) for jb in range(cfg.jbn) for mt in range(cfg.mt)]
            pw_pos = [0]

            def emit_pw(n):
                for _ in range(n):
                    if pw_pos[0] < len(pw_units):
                        pw_unit(*pw_units[pw_pos[0]])
                        pw_pos[0] += 1

            # ---- NEG phase: row gathers + fp16 sub + split reduce,
            #      pairwise units woven between chunks
            negacc = const.tile([P, cfg.rc, cfg.m], f32)
            per_chunk = -(-len(pw_units) // nchunks)
            for rcb in range(cfg.rc):
                q2s = q2_sb[:, rcb, :]
                q2bc = bass.AP(
                    q2s.tensor, q2s.offset,
                    [q2s.ap[0], [0, cfg.mg], q2s.ap[1]])  # [P, mg, d] bcast
                for g in range(cfg.ng):
                    c = rcb * cfg.ng + g
                    ms = slice(g * cfg.mg, (g + 1) * cfg.mg)
                    at = work.tile([P, cfg.mg, cfg.d], f16, tag="negload")
                    for j in range(cfg.mg):
                        mj = g * cfg.mg + j
                        nc.gpsimd.indirect_dma_start(
                            out=at[:, j, :], out_offset=None, in_=embS16_d[:],
                            in_offset=IOA(ap=negidx_sb[:, rcb, mj:mj + 1], axis=0))
                    diff = dwork.tile([P, cfg.mg, cfg.d], f16, tag="diff")
                    nc.vector.tensor_sub(diff[:], at[:], q2bc)
                    if c in act_set:
                        trash = twork.tile([P, cfg.mg, cfg.d], f16, tag="trash")
                        for j in range(cfg.mg):
                            mj = g * cfg.mg + j
                            nc.scalar.activation(
                                out=trash[:, j, :], in_=diff[:, j, :], func=ABS,
                                accum_out=negacc[:, rcb, mj:mj + 1])
                    else:
                        nc.vector.tensor_reduce(
                            negacc[:, rcb, ms], diff[:],
                            axis=AX.X, op=AL.add, apply_absolute_value=True)
                    emit_pw(per_chunk)
            emit_pw(len(pw_units))
            nc.sync.dma_start(oneg_d[:], negacc[:])
            for mt in range(cfg.mt):
                nc.vector.tensor_reduce(
                    ointer_sb[:, mt:mt + 1], jmall[:, mt, :], axis=AX.X, op=AL.min)
            nc.sync.dma_start(ointer_d[:], ointer_sb[:])

    nc.compile()
    return nc


def _chunked(x, nch):
    """[N, ...] -> [128, nch, ...] with row r at [r%128, r//128]."""
    n = x.shape[0]
    assert n == nch * P
    return np.ascontiguousarray(x.reshape(nch, P, *x.shape[1:]).transpose(
        1, 0, *range(2, x.ndim + 1)))


def make_in_maps(cfg: Cfg, sample, neg_tails, cluster_assign, parent_assign,
                 relation_embedding, entity_embedding_init,
                 entity_text_embeddings, cluster_emb, parent_emb):
    f4 = np.float32
    sample = np.asarray(sample)
    neg_tails = np.asarray(neg_tails)
    cluster_assign = np.asarray(cluster_assign)
    parent_assign = np.asarray(parent_assign)
    relation_embedding = np.asarray(relation_embedding, dtype=f4)
    embA = np.asarray(entity_embedding_init, dtype=f4)
    embT = np.asarray(entity_text_embeddings, dtype=f4)
    embS = embA + embT          # = 2 * comb
    embS16 = embS.astype(F16)
    cluster_emb = np.asarray(cluster_emb, dtype=f4)
    parent_emb = np.asarray(parent_emb, dtype=f4)

    h_all = sample[:, 0].astype(np.int64)
    r_all = (sample[:, 1] % cfg.nrel).astype(np.int64)
    t_all = sample[:, 2].astype(np.int64)
    ht_all = np.concatenate([h_all, t_all])
    cid_all = cluster_assign[ht_all]
    clus = cluster_emb[cid_all]                       # [2B, d] f32
    clus16 = clus.astype(F16)                        # device-consistent rounding
    clus16f = clus16.astype(f4)
    sq_all = np.sum(clus16f * clus16f, axis=1, dtype=f4)   # [2B] from bf16 vals
    sq_hi = sq_all.astype(F16)
    sq_lo = (sq_all - sq_hi.astype(f4)).astype(F16)
    pars_all = parent_emb[parent_assign[cid_all]]
    eye = (np.eye(P) * 1e9).astype(f4)

    # ---- host-side per-row score terms (exact f32)
    Sh, St = embS[h_all], embS[t_all]
    rel = relation_embedding[r_all]
    true_s = (GAMMA - np.abs(0.5 * Sh + rel - 0.5 * St).sum(axis=1, dtype=f4)
              ).astype(f4)                                        # [B]
    embD_ht = embA[ht_all] - embT[ht_all]
    text_d = np.sqrt(0.25 * np.sum(embD_ht * embD_ht, axis=1, dtype=f4) + EPS)
    intra_v = 0.5 * embS[ht_all] - clus
    intra_d = np.sqrt(np.sum(intra_v * intra_v, axis=1, dtype=f4) + EPS)
    par_v = clus - pars_all
    par_d = np.sqrt(np.sum(par_v * par_v, axis=1, dtype=f4) + EPS)
    host = {
        "true_s": true_s,
        "hd": text_d[:cfg.b].astype(f4),
        "td": text_d[cfg.b:].astype(f4),
        "intra_loss": intra_d.mean(dtype=f4),
        "par_loss": par_d.mean(dtype=f4),
        "sq_all": sq_all,
    }

    in_maps = []
    perms = []
    for k in range(cfg.ncores):
        bs = slice(k * cfg.pc, (k + 1) * cfg.pc)
        h = h_all[bs]
        r = r_all[bs]
        neg = neg_tails[bs].astype(np.int32)          # [pc, m]
        negidx = _chunked(neg, cfg.rc)
        q2 = (embS[h] + 2.0 * relation_embedding[r]).astype(F16)
        q2_in = _chunked(q2, cfg.rc)

        own = np.concatenate([np.arange(k * cfg.pc, (k + 1) * cfg.pc),
                              np.arange(cfg.b + k * cfg.pc,
                                        cfg.b + (k + 1) * cfg.pc)])
        mask = np.ones(cfg.nall, dtype=bool)
        mask[own] = False
        perm = np.concatenate([own, np.nonzero(mask)[0]])
        perms.append(perm)
        clusP = clus16[perm]                          # [nall, d] bf16
        clusT_in = np.ascontiguousarray(
            clusP.T.reshape(cfg.kc, P, cfg.nall).transpose(1, 0, 2))
        lhs2_in = np.ascontiguousarray(
            (-2.0 * clusP[:cfg.hr].astype(f4)).astype(F16)
            .T.reshape(cfg.kc, P, cfg.hr).transpose(1, 0, 2))
        ones2 = np.ones((2, cfg.hr), dtype=F16)
        sqrows = np.stack([sq_hi[perm], sq_lo[perm]]).astype(F16)

        in_maps.append({
            "embS16": embS16,
            "negidx": negidx.astype(np.int32),
            "q2": q2_in,
            "clusT": clusT_in,
            "lhs2": lhs2_in,
            "ones2": ones2,
            "sqrows": sqrows,
            "eye": eye,
        })
    return in_maps, (host, perms)


def _unchunk(x):
    """[128, nch, ...] -> [nch*128, ...] inverting _chunked."""
    return np.ascontiguousarray(
        x.transpose(1, 0, *range(2, x.ndim))).reshape(-1, *x.shape[2:])


def assemble(cfg: Cfg, results, aux):
    host, perms = aux
    f4 = np.float32
    mean_neg, inter_d2 = [], np.empty(cfg.nall, dtype=f4)
    for k in range(cfg.ncores):
        r = results[k]
        raw_neg = _unchunk(r["o_neg"])                # [pc, m]
        neg_scores = (GAMMA - 0.5 * raw_neg).astype(f4)
        mean_neg.append(neg_scores.mean(axis=1, dtype=f4))
        own = perms[k][:cfg.hr]
        inter_min = _unchunk(r["o_inter"][:, :, None])[:, 0]   # [hr]
        inter_d2[own] = inter_min + host["sq_all"][own]
    mean_neg = np.concatenate(mean_neg)

    inter_d = np.sqrt(np.maximum(inter_d2, EPS), dtype=f4)
    inter_loss = inter_d.mean(dtype=f4)
    hier = host["intra_loss"] - LAM1 * inter_loss + LAM2 * host["par_loss"]

    score = (-ALPHA * hier - BETA * (host["hd"] + host["td"])
             - GAMMA_2 * (host["true_s"] - mean_neg)).astype(f4)
    return score


def run_on_device(cfg: Cfg, in_maps, trace=False):
    from concourse.bass_utils import run_bass_kernel_spmd
    key = cfg
    if key not in _PROG_CACHE:
        _PROG_CACHE[key] = build_program(cfg)
    nc = _PROG_CACHE[key]
    res = run_bass_kernel_spmd(
        nc, in_maps, core_ids=list(range(cfg.ncores)), trace=trace)
    return res


def kernel(**inputs):
    cfg = REAL
    in_maps, aux = make_in_maps(cfg, **inputs)
    res = run_on_device(cfg, in_maps)
    return assemble(cfg, res.results, aux)
